# revision 1
# baseline (speedup 1.0000x reference)
"""Additive (Bahdanau-style) attention on 8 Trainium2 NeuronCores.

Math: scores[b,q,k] = Wt . tanh(u[b,k] + v[b,q]) + bt, masked softmax over k,
out = weights @ hidden.  (bt is dropped: softmax is shift-invariant.)

Key trick: tanh(x) on the data range |x| <= 10.4 is approximated by a
free-frequency sine expansion  tanh(x) ~= sum_m beta_m * sin(om_m * x)
(sup err 1.7e-4, end-to-end rel err ~6e-5).  The angle-addition identity
  sin(om(u+v)) = sin(om u)cos(om v) + cos(om u)sin(om v)
factorizes the [Sq,Sk,A] tanh tensor into per-(a,k) / per-(a,q) feature maps
plus PE matmuls contracting over A - the O(S^2 A) elementwise work never
exists.  Sin args are range-reduced to [-pi, pi] with a fused custom DVE op
(frac via the magic-constant rounding trick), since the ACT Sin table is only
accurate for |arg| <= ~3.3.

Sharding: core c -> batch b = c//2, query half qoff = (c%2)*256.  Each core's
"hidden" input is rolled by -qoff so queries are always rows 0..255 (pure SPMD,
no per-core program differences); a key permutation is softmax-invariant as
long as mask and values are permuted consistently.
"""

import numpy as np

import concourse.bass as bass
import concourse.tile as tile
from concourse import bacc, mybir
from concourse.bass_utils import run_bass_kernel_spmd

# ---- problem constants (hardcoded; kernel.py must be self-contained) -------
B, S, D, A = 4, 512, 256, 128
QPC = 256          # queries per core
NCORES = 8
MASK_NEG = -30000.0
MAGIC = float(1.5 * 2 ** 23)     # fp32 round-to-nearest magic constant
TWO_PI = float(2.0 * np.pi)

# ---- tanh ~= sum_m beta_m sin(om_m x) on [-10.4, 10.4], sup err 1.70e-4 ----
OMEGA = [
    0.2598461821833046,
    0.5344304542355706,
    0.7790641695425123,
    1.0447475699402693,
    1.3059865501613794,
    1.5676866780364966,
    1.8291014889529025,
    2.0815890485254016,
    2.335103390013457,
    2.596640471895237,
    2.8535760956755007,
    3.1062103574560376,
    3.3843764770401843,
    3.9818154717464087,
    4.565796965524829,
    5.152592467288602,
]
BETA = [
    1.2472679595576082,
    -0.017333216720805067,
    0.3560118857421656,
    -0.02058564456660658,
    0.1613001438864655,
    -0.024672915541359766,
    0.08277285691296736,
    -0.02130757248816391,
    0.041216358641381616,
    -0.009765648710735613,
    0.017819489756314477,
    -0.0030220436306928823,
    0.00638370829919329,
    0.002244044100905601,
    0.0008990662865403671,
    0.00042650834190054263,
]
M = len(OMEGA)
GROUP = 4                        # max frequencies per pipeline group
GROUP_SIZES = [4, 4, 4, 2, 2]    # tapered tail: shorter final Sin+scale+mm chain
assert sum(GROUP_SIZES) == M
N_GROUPS = len(GROUP_SIZES)

TRACE = False                    # test.py sets True for the profiled run
LAST_EXEC_NS = None


def _ensure_ntff_hook():
    """The agent image's `antenv` lacks `axon_hooks`, so the boot-time NTFF
    hook registration silently degrades.  Recreate it: install a stub module
    and wire it to the ctypes profiler in trn_agent_boot."""
    import sys, types
    if "antenv.axon_hooks" in sys.modules:
        return
    mod = types.ModuleType("antenv.axon_hooks")
    _h = [None]
    mod.set_axon_ntff_profile_hook = lambda h: _h.__setitem__(0, h)
    mod.get_axon_ntff_profile_hook = lambda: _h[0]
    import antenv
    sys.modules["antenv.axon_hooks"] = mod
    antenv.axon_hooks = mod
    try:
        from trn_agent_boot.trn_boot import _ntff_profile_via_ctypes
        mod.set_axon_ntff_profile_hook(
            _ntff_profile_via_ctypes("/opt/axon/libaxon_pjrt.so"))
    except Exception:
        pass

# ---- custom DVE ops ---------------------------------------------------------
# FRAC_AFFINE_ATT:   out = t - round(t),  t = in0*s0 + s1
# FRAC_PAGED_ATT:    out[p, pg, k] = t - round(t),
#                    t = in0*s0 + in1 + pg*s1   (pg = page index 0/1)
_FRAC_OP = None
_FRAC_PAGED_OP = None


def _frac_paged_reference(in0, in1, s0, s1, imm2):
    f32 = np.float32
    pg = np.arange(in0.shape[1], dtype=f32)[None, :, None] * f32(s1)
    t = (in0.astype(f32) * f32(s0) + in1.reshape(-1, 1, 1).astype(f32)
         + pg).astype(f32)
    r = ((t + f32(imm2)).astype(f32) - f32(imm2)).astype(f32)
    return (t - r).astype(f32)


def _get_frac_paged_op():
    global _FRAC_PAGED_OP
    if _FRAC_PAGED_OP is not None:
        return _FRAC_PAGED_OP
    from concourse import dve_ops as dvo
    from concourse.dve_spec import (C0, C1, C2, PageIdx, Spec, Src0, Src1,
                                    Zero, lower, _has_src1)
    from concourse.dve_uop import DveOpSpec

    name = "FRAC_PAGED_ATT"
    for op in dvo.OPS:
        if op.name == name:
            _FRAC_PAGED_OP = op
            return op
    t = Src0 * C0 + Src1 + PageIdx(Zero, C1)
    spec = Spec(body=t - ((t + C2) - C2), reference=_frac_paged_reference)
    op = dvo.DveOp(name, spec, subdim=True, uops_sha={})
    dvo.OPS.append(op)
    dvo.CUSTOM_DVE_SPECS[name] = spec
    dvo._SUB_OPCODE_FOR_NAME[name] = max(dvo._SUB_OPCODE_FOR_NAME.values()) + 1
    assert dvo._SUB_OPCODE_FOR_NAME[name] < 0x20
    for ver in ("v3", "v4"):
        compiled = DveOpSpec(
            name=name,
            opcode=dvo.get_dve_sub_opcode(name),
            uops=lower(spec, ver=ver),
            rd1_en=_has_src1(spec),
        )
        op.uops_sha[ver] = compiled.sha(ver)
    _FRAC_PAGED_OP = op
    return op


def _frac_reference(in0, in1, s0, s1, imm2):
    f32 = np.float32
    t = (in0.astype(f32) * f32(s0) + f32(s1)).astype(f32)
    r = ((t + f32(imm2)).astype(f32) - f32(imm2)).astype(f32)
    return (t - r).astype(f32)


def _get_frac_op():
    global _FRAC_OP
    if _FRAC_OP is not None:
        return _FRAC_OP
    from concourse import dve_ops as dvo
    from concourse.dve_spec import C0, C1, C2, Spec, Src0, lower, _has_src1
    from concourse.dve_uop import DveOpSpec

    name = "FRAC_AFFINE_ATT"
    for op in dvo.OPS:
        if op.name == name:
            _FRAC_OP = op
            return op
    t = Src0 * C0 + C1
    spec = Spec(body=t - ((t + C2) - C2), reference=_frac_reference)
    op = dvo.DveOp(name, spec, subdim=False, uops_sha={})
    dvo.OPS.append(op)
    dvo.CUSTOM_DVE_SPECS[name] = spec
    dvo._SUB_OPCODE_FOR_NAME[name] = max(dvo._SUB_OPCODE_FOR_NAME.values()) + 1
    assert dvo._SUB_OPCODE_FOR_NAME[name] < 0x20
    for ver in ("v3", "v4"):
        compiled = DveOpSpec(
            name=name,
            opcode=dvo.get_dve_sub_opcode(name),
            uops=lower(spec, ver=ver),
            rd1_en=_has_src1(spec),
        )
        op.uops_sha[ver] = compiled.sha(ver)
    _FRAC_OP = op
    return op


# ---- device program --------------------------------------------------------
_NC = None


def _build_program():
    frac = _get_frac_op()
    f32 = mybir.dt.float32
    nc = bacc.Bacc("TRN2", target_bir_lowering=False, debug=False,
                   num_devices=NCORES)

    h_ext = nc.dram_tensor("h", [S, D], f32, kind="ExternalInput").ap()
    ht_ext = nc.dram_tensor("ht", [D, S], f32, kind="ExternalInput").ap()
    wut_ext = nc.dram_tensor("wut", [D, A], f32, kind="ExternalInput").ap()
    wvt_ext = nc.dram_tensor("wvt", [D, A], f32, kind="ExternalInput").ap()
    mb_ext = nc.dram_tensor("mb", [1, S], mybir.dt.float16, kind="ExternalInput").ap()
    wu_ext = nc.dram_tensor("wu", [A, D], f32, kind="ExternalInput").ap()
    wv_ext = nc.dram_tensor("wv", [A, D], f32, kind="ExternalInput").ap()
    wt_ext = nc.dram_tensor("wt", [A, 1], f32, kind="ExternalInput").ap()
    bu_ext = nc.dram_tensor("bu", [A, 1], f32, kind="ExternalInput").ap()
    out_ext = nc.dram_tensor("out", [QPC, D], f32, kind="ExternalOutput").ap()

    from concourse.masks import make_identity
    P = 128
    SIN = mybir.ActivationFunctionType.Sin
    EXP = mybir.ActivationFunctionType.Exp
    ALU = mybir.AluOpType

    with tile.TileContext(nc) as tc:
        import contextlib
        with contextlib.ExitStack() as ctx:
            const = ctx.enter_context(tc.tile_pool(name="const", bufs=1))
            upool = ctx.enter_context(tc.tile_pool(name="upool", bufs=3))
            vpool = ctx.enter_context(tc.tile_pool(name="vpool", bufs=3))
            wpool = ctx.enter_context(tc.tile_pool(name="wpool", bufs=2))
            stat = ctx.enter_context(tc.tile_pool(name="stat", bufs=4))
            pp_scores = ctx.enter_context(
                tc.tile_pool(name="pp_scores", bufs=2, space="PSUM"))
            pp_work = ctx.enter_context(
                tc.tile_pool(name="pp_work", bufs=1, space="PSUM"))
            pp_tr = ctx.enter_context(
                tc.tile_pool(name="pp_tr", bufs=2, space="PSUM"))

            # ---- constants & inputs ----
            identity = const.tile([P, P], f32)
            make_identity(nc, identity)
            ones1 = const.tile([1, P], mybir.dt.float16)
            nc.vector.memset(ones1, 1.0)
            zbias = const.tile([P, 1], f32)
            nc.vector.memset(zbias, 0.0)

            wvT = const.tile([P, 2, P], f32)
            nc.sync.dma_start(out=wvT,
                              in_=wvt_ext.rearrange("(c p) a -> p c a", p=P))
            wuT = const.tile([P, 2, P], f32)
            bu_sb = const.tile([P, 1], f32)
            nc.scalar.dma_start(out=bu_sb, in_=bu_ext[:])
            wt_sb = const.tile([P, 1], f32)
            nc.scalar.dma_start(out=wt_sb, in_=wt_ext[:])
            f16 = mybir.dt.float16
            mb_sb = const.tile([1, S], f16)
            nc.scalar.dma_start(out=mb_sb, in_=mb_ext[:])
            h_sb = const.tile([P, 4, D], f32)       # rows s = t*128+p
            h_r = h_ext.rearrange("(t p) d -> p t d", p=P)

            # beta_m * Wt columns (scale applied to the fp16 u-side maps)
            bwt = const.tile([P, M], f32)
            for m in range(M):
                nc.gpsimd.tensor_scalar(
                    out=bwt[:, m:m + 1], in0=wt_sb, scalar1=float(BETA[m]),
                    scalar2=None, op0=ALU.mult)
            # FRAC bias vectors for the v side: bu*s_m + phi
            bu_s = const.tile([P, 2 * M], f32)
            for m in range(M):
                s_m = float(OMEGA[m] / TWO_PI)
                for ph, phi in ((0, 0.0), (1, 0.25)):
                    nc.gpsimd.tensor_scalar(
                        out=bu_s[:, 2 * m + ph:2 * m + ph + 1], in0=bu_sb,
                        scalar1=s_m, scalar2=phi,
                        op0=ALU.mult, op1=ALU.add)

            # ---- hT from host; projections fire as soon as slices land ----
            hT = const.tile([P, 2, S], f32)
            ht_r = ht_ext.rearrange("(c p) s -> p c s", p=P)
            # v-projection needs only s<256 columns: DMA those first
            for c in range(2):
                nc.sync.dma_start(out=hT[:, c, 0:QPC], in_=ht_r[:, c, 0:QPC])
            psum_v = pp_work.tile([P, QPC], f32, tag="pv")
            for c in range(2):
                nc.tensor.matmul(psum_v, wvT[:, c, :], hT[:, c, 0:QPC],
                                 start=(c == 0), stop=(c == 1))
            nc.sync.dma_start(out=wuT,
                              in_=wut_ext.rearrange("(c p) a -> p c a", p=P))
            for c in range(2):
                nc.sync.dma_start(out=hT[:, c, QPC:S], in_=ht_r[:, c, QPC:S])
            for t in range(4):
                nc.scalar.dma_start(out=h_sb[:, t, :], in_=h_r[:, t, :])
            psum_u = pp_work.tile([P, S], f32, tag="pu")
            for c in range(2):
                nc.tensor.matmul(psum_u, wuT[:, c, :], hT[:, c, :],
                                 start=(c == 0), stop=(c == 1))

            # ---- scores psum, seeded with the additive key mask ----
            ps_scores = []
            for qb in range(2):
                ps = pp_scores.tile([P, 2 * S], f32)
                nc.tensor.matmul(ps[:, 0:S], ones1, mb_sb, start=True,
                                 stop=False)
                ps_scores.append(ps)

            # PE "heater": garbage matmuls into the unused upper half of the
            # scores psum tiles keep the HAM busy-window alive while the PE
            # waits for feature maps (idle >3.4us re-throttles it to 1.2GHz).
            # Each heater reads a tile the current pipeline stage just wrote,
            # so the scheduler places it exactly inside the wait.
            def heat(n, dep):
                w = min(dep.shape[-1], P)
                for i in range(n):
                    nc.tensor.matmul(ps_scores[i % 2][0:w, S + 256:S + 256 + w],
                                     dep[:, 0:w], dep[:, 0:w],
                                     start=False, stop=False,
                                     skip_group_check=True)


            # ---- feature groups: FRAC -> Sin(fp16) -> scale -> matmuls ----
            pending = []

            def flush(last_g):
                ms_, um_, vm_ = pending.pop(0)
                ng_ = len(ms_)
                for j, m in enumerate(ms_):
                    nc.vector.tensor_scalar(
                        out=um_[:, j, :, :], in0=um_[:, j, :, :],
                        scalar1=bwt[:, m:m + 1], scalar2=None, op0=ALU.mult)
                for qb in range(2):
                    qs = slice(qb * P, (qb + 1) * P)
                    for j, m in enumerate(ms_):
                        last = last_g and (j == ng_ - 1)
                        nc.tensor.matmul(ps_scores[qb][:, 0:S],
                                         vm_[:, j, 1, qs], um_[:, j, 0, :],
                                         start=False, stop=False)
                        nc.tensor.matmul(ps_scores[qb][:, 0:S],
                                         vm_[:, j, 0, qs], um_[:, j, 1, :],
                                         start=False, stop=last)

            g_start = 0
            for g in range(N_GROUPS):
                ms = list(range(g_start, g_start + GROUP_SIZES[g]))
                g_start += GROUP_SIZES[g]
                ng = len(ms)
                uf = upool.tile([P, GROUP, 2, S], f32, tag="uf")
                vf = vpool.tile([P, GROUP, 2, QPC], f32, tag="vf")
                um = upool.tile([P, GROUP, 2, S], f16, tag="um")
                vm = vpool.tile([P, GROUP, 2, QPC], f16, tag="vm")
                for j, m in enumerate(ms):
                    s_m = float(OMEGA[m] / TWO_PI)
                    for ph, phi in ((0, 0.0), (1, 0.25)):
                        nc.vector._custom_dve(
                            frac, out=vf[:, j, ph, :], in0=psum_v,
                            s0=s_m, s1=bu_s[:, 2 * m + ph:2 * m + ph + 1],
                            imm2=MAGIC)
                        nc.vector._custom_dve(
                            frac, out=uf[:, j, ph, :], in0=psum_u,
                            s0=s_m, s1=phi, imm2=MAGIC)
                nc.scalar.activation(vm[:, 0:ng, :, :], vf[:, 0:ng, :, :],
                                     SIN, bias=zbias, scale=TWO_PI)
                nc.scalar.activation(um[:, 0:ng, :, :], uf[:, 0:ng, :, :],
                                     SIN, bias=zbias, scale=TWO_PI)

                # scales + score matmuls run one group behind the FRAC/Sin
                # stage: by the time the DVE reaches a group's scale ops, that
                # group's Sin has long finished, so the (FIFO) DVE stream
                # never stalls behind a not-yet-ready instruction.
                pending.append((ms, um, vm))
                if g > 0:
                    flush(last_g=False)
            flush(last_g=True)

            # ---- masked softmax + output ----
            for qb in range(2):
                w_sb = wpool.tile([P, S], f32, tag="w")
                sums = stat.tile([P, 1], f32, tag="sums")
                nc.scalar.activation(w_sb, ps_scores[qb][:, 0:S], EXP, bias=zbias,
                                     scale=1.0, accum_out=sums)
                rsum = stat.tile([P, 1], f32, tag="rsum")
                nc.vector.reciprocal(rsum, sums)
                wT = wpool.tile([P, 4, P], f32, tag="wT")
                for kc in range(4):
                    pt = pp_tr.tile([P, P], f32)
                    nc.tensor.transpose(pt, w_sb[:, kc * P:(kc + 1) * P],
                                        identity)
                    nc.scalar.copy(wT[:, kc, :], pt)
                ps_o = pp_work.tile([P, D], f32, tag="pu")
                for kc in range(4):
                    nc.tensor.matmul(ps_o, wT[:, kc, :], h_sb[:, kc, :],
                                     start=(kc == 0), stop=(kc == 3))
                out_sb = wpool.tile([P, D], f32, tag="out")
                nc.scalar.mul(out_sb, ps_o, rsum)
                nc.sync.dma_start(out=out_ext[qb * P:(qb + 1) * P, :],
                                  in_=out_sb)

    nc.compile()
    return nc


def kernel(hidden, mask, Wu, bu, Wv, Wt, bt):
    global _NC, LAST_EXEC_NS
    if _NC is None:
        _NC = _build_program()
    nc = _NC

    hidden = np.asarray(hidden, dtype=np.float32)
    mask = np.asarray(mask)
    Wu = np.ascontiguousarray(np.asarray(Wu, dtype=np.float32))
    Wv = np.ascontiguousarray(np.asarray(Wv, dtype=np.float32))
    Wt_c = np.ascontiguousarray(np.asarray(Wt, dtype=np.float32).reshape(A, 1))
    WuT_c = np.ascontiguousarray(Wu.T)
    WvT_c = np.ascontiguousarray(Wv.T)
    bu_c = np.ascontiguousarray(np.asarray(bu, dtype=np.float32).reshape(A, 1))

    in_maps = []
    for c in range(NCORES):
        b, half = divmod(c, 2)
        qoff = half * QPC
        hr = np.ascontiguousarray(np.roll(hidden[b], -qoff, axis=0))
        mb = np.where(np.asarray(mask[b]) < 1, MASK_NEG, 0.0).astype(np.float16)
        mbr = np.ascontiguousarray(np.roll(mb, -qoff).reshape(1, S))
        in_maps.append({"h": hr, "ht": np.ascontiguousarray(hr.T),
                        "mb": mbr, "wu": Wu, "wv": Wv,
                        "wut": WuT_c, "wvt": WvT_c,
                        "wt": Wt_c, "bu": bu_c})

    if TRACE:
        _ensure_ntff_hook()
    res = run_bass_kernel_spmd(nc, in_maps, list(range(NCORES)), trace=TRACE)
    LAST_EXEC_NS = res.exec_time_ns

    out = np.empty((B, S, D), dtype=np.float32)
    for c in range(NCORES):
        b, half = divmod(c, 2)
        qoff = half * QPC
        out[b, qoff:qoff + QPC] = res.results[c]["out"]
    return out

